# revision 33
# baseline (speedup 1.0000x reference)
"""Trainium2 Bass kernel for the LeNet C3 dense-conv layer.

Computes out = conv2d_valid(x, K, stride 1) + bias where K is the dense
[16, 6, 5, 5] kernel scattered from the sparse per-branch weights
(w3/w4/w6), x is [128, 6, 256, 256] f32, out is [128, 16, 252, 252] f32.

Strategy:
  - Pure data parallelism: 16 images per NeuronCore across 8 cores.
  - The conv is computed as shift-accumulated banded matmuls into PSUM.
    A block covers 6 output rows of ALL 16 images: the contraction dim
    stacks TWO copies of the 10 input rows (60 partitions each), the
    second copy pre-shifted one column, so each matmul covers two kernel
    columns kx at once: 3 matmuls per image pair (kx {0,1}, {2,3}, {4})
    instead of 5. The lhsT is a host-built banded matrix [120, 96] whose
    column m = c_out*6 + r holds K_dense[c_out, c_in, i-r, kx] at row
    i*6 + c_in (+60 for the kx+1 band). K = 120 > 96 keeps all four PE
    row-group quarters active (row_grp=0xf), required for the full
    1-column/cycle stream rate (K <= 96 configs measure at half rate).
    M = 96 != 128 avoids fast-weight-load, whose 4-XBUS weight fetch
    also halves the stream rate when weights swap per matmul.
  - The host pre-builds each block's ENTIRE input tile (both stacked
    copies, pads zeroed) in DRAM, so each block needs a single 985 KB
    input DMA with 8.2 KB descriptors and 120 outer units (perfectly
    striped over all 16 SDMA engines). HWDGE rings cost ~9 ns per
    descriptor, so few large DMAs win.
  - Each matmul's moving operand is an image pair (N = 512 = one PSUM
    bank); four pairs accumulate into one 4-bank PSUM tile per 8-image
    sub-round, evicted by a single vector-engine op with the bias add
    fused, then written by one 774 KB output DMA (8 KB descriptors).
  - fp16 operands (~3e-4 rel err; accumulation is fp32 in PSUM).
  - Host packs/unpacks the interleaved layouts (x fp16 cast + block
    stacking; output o8[oc, c, h, j*252+w] -> NCHW).
"""

import numpy as np

# LeNet-5 C3 sparse channel connectivity (from the model definition).
CH3 = np.array([[0, 1, 2], [1, 2, 3], [2, 3, 4], [3, 4, 5], [0, 4, 5], [0, 1, 5]])
CH4 = np.array([[0, 1, 2, 3], [1, 2, 3, 4], [2, 3, 4, 5], [0, 3, 4, 5],
                [0, 1, 4, 5], [0, 1, 2, 5], [0, 1, 3, 4], [1, 2, 4, 5],
                [0, 2, 3, 5]])

B, C, H, W = 128, 6, 256, 256
CO, HO, WO = 16, 252, 252
NCORES = 8
BPC = B // NCORES           # images per core (16)
KH = KW = 5

R = 6                       # output rows per block
HI = R + 4                  # input rows per block (10)
NBLK = HO // R              # 42 blocks
KK = C * HI                 # contraction rows per kx copy (60)
MM = CO * R                 # psum partitions (96)
TW = 4 + BPC * W            # input tile width (4100)

_STATE = None  # cached Bass module so repeat kernel() calls skip re-tracing


def _dense_kernel(w3, w4, w6):
    k = np.zeros((CO, C, KH, KW), np.float32)
    k[np.arange(6)[:, None], CH3] = w3
    k[6 + np.arange(9)[:, None], CH4] = w4
    k[15] = w6[0]
    return k


def _band(kd, kx):
    """Banded lhsT [KK, MM] for kernel column kx: row i*6 + c_in,
    column c_out*R + r, value kd[c_out, c_in, i-r, kx]."""
    out = np.zeros((KK, MM), np.float32)
    for ci in range(C):
        for i in range(HI):
            for r in range(R):
                ky = i - r
                if 0 <= ky < KH:
                    out[i * C + ci, np.arange(CO) * R + r] = kd[:, ci, ky, kx]
    return out


def _build_module():
    import concourse.bacc as bacc
    import concourse.mybir as mybir
    from concourse.tile import TileContext

    f32 = mybir.dt.float32
    f16 = mybir.dt.float16

    # Bacc (not Bass): its compile() runs generate_event_semaphores(),
    # which splits multi-wait instructions to satisfy the TRN2 1-wait-
    # per-instruction constraint walrus enforces.
    nc = bacc.Bacc(None)
    # Pre-stacked per-block input tiles (see module docstring).
    x_d = nc.dram_tensor("x", [NBLK, 2 * KK, TW], f16, kind="ExternalInput")
    # wall: [120, 3*96] = [B(0); B(1)] | [B(2); B(3)] | [B(4); 0]
    wall_d = nc.dram_tensor("wall", [2 * KK, 3 * MM], f16, kind="ExternalInput")
    b1_d = nc.dram_tensor("b1", [MM, 1], f32, kind="ExternalInput")
    # o8[oc, c, h, j*252 + w] = out[8*oc + j, c, h, w]  (host un-packs).
    # Staged in fp16 to halve the dominant output DMA stream; the host
    # casts back to f32 (adds ~5e-4 relative quantization).
    o_d = nc.dram_tensor("o", [2, CO, HO, 8 * WO], f16, kind="ExternalOutput")

    with TileContext(nc) as tc:
        with (
            tc.tile_pool(name="wpool", bufs=1) as wp,
            tc.tile_pool(name="inpool", bufs=6) as ip,
            tc.tile_pool(name="outpool", bufs=8) as op,
            tc.tile_pool(name="pspool", bufs=2, space="PSUM") as pp,
        ):
            wall_t = wp.tile([2 * KK, 3 * MM], f16)
            nc.sync.dma_start(wall_t[:], wall_d[:])
            b1_t = wp.tile([MM, 1], f32)
            nc.sync.dma_start(b1_t[:], b1_d[:])

            # Prime the constant tiles on their consuming engine classes so
            # steady-state instructions carry few semaphore waits.
            prime_ps = pp.tile([MM, 192], f32, tag="ps")
            nc.tensor.matmul(prime_ps[:], wall_t[:, 0:MM], wall_t[:, 0:192],
                             start=True, stop=True)
            prime_b = op.tile([MM, 1], f16, tag="out")
            nc.vector.tensor_scalar_add(prime_b[:], b1_t[:], 0.0)

            for g in range(NBLK):
                h0 = R * g
                it = ip.tile([2 * KK, TW], f16, tag="in")
                nc.sync.dma_start(it[:, :], x_d[g])

                for sr in range(2):             # 8-image sub-rounds
                    # One 4-bank PSUM tile holds four image pairs.
                    ps = pp.tile([MM, 8, 256], f32, tag="ps")
                    for grp in range(4):
                        b = 2048 * sr + 512 * grp
                        pslice = ps[:, 2 * grp:2 * grp + 2, :]
                        # offsets 0/2/4 -> kx {0,1} / {2,3} / {4}
                        nc.tensor.matmul(pslice, wall_t[:, 0:MM],
                                         it[:, b:b + 512],
                                         start=True, stop=False)
                        nc.tensor.matmul(pslice, wall_t[:, MM:2 * MM],
                                         it[:, b + 2:b + 514],
                                         start=False, stop=False)
                        nc.tensor.matmul(pslice, wall_t[:, 2 * MM:3 * MM],
                                         it[:, b + 4:b + 516],
                                         start=False, stop=True)

                    # Single eviction per sub-round, bias fused, fp16 cast.
                    ot = op.tile([MM, 8 * WO], f16, tag="out")
                    nc.vector.tensor_scalar_add(
                        ot[:].rearrange("p (j w) -> p j w", j=8),
                        ps[:, :, 4:4 + WO],
                        b1_t[:, 0:1],
                    )
                    nc.scalar.dma_start(o_d[sr, :, h0:h0 + R, :], ot[:])
    nc.compile()
    return nc


def _get_module():
    global _STATE
    if _STATE is None:
        _STATE = _build_module()
    return _STATE


def kernel(x, w3, b3, w4, b4, w6, b6):
    from concourse.bass_utils import run_bass_kernel_spmd

    x = np.asarray(x, np.float32)
    kd = _dense_kernel(np.asarray(w3, np.float32), np.asarray(w4, np.float32),
                       np.asarray(w6, np.float32))
    bias = np.concatenate([np.asarray(b3, np.float32),
                           np.asarray(b4, np.float32),
                           np.asarray(b6, np.float32)])

    zero = np.zeros((KK, MM), np.float32)
    wall = np.concatenate([
        np.concatenate([_band(kd, 0), _band(kd, 2), _band(kd, 4)], axis=1),
        np.concatenate([_band(kd, 1), _band(kd, 3), zero], axis=1),
    ], axis=0).astype(np.float16)
    b1 = np.repeat(bias, R).astype(np.float32).reshape(MM, 1)

    nc = _get_module()
    x16 = x.astype(np.float16)
    in_maps = []
    for cr in range(NCORES):
        xs = x16[cr * BPC:(cr + 1) * BPC]
        # rows[(h, c), j*256 + w] = x[j, c, h, w]
        rows = np.ascontiguousarray(
            xs.transpose(2, 1, 0, 3)).reshape(H * C, BPC * W)
        xstk = np.zeros((NBLK, 2 * KK, TW), np.float16)
        for g in range(NBLK):
            blk = rows[R * C * g: R * C * g + KK]
            xstk[g, 0:KK, 4:4 + BPC * W] = blk
            xstk[g, KK:2 * KK, 3:3 + BPC * W] = blk
        in_maps.append({"x": xstk, "wall": wall, "b1": b1})
    res = run_bass_kernel_spmd(nc, in_maps, core_ids=list(range(NCORES)))
    global LAST_RESULT
    LAST_RESULT = res

    out = np.empty((B, CO, HO, WO), np.float32)
    for cr in range(NCORES):
        o8 = res.results[cr]["o"].astype(np.float32).reshape(2, CO, HO, 8, WO)
        out[cr * BPC:(cr + 1) * BPC] = (
            o8.transpose(0, 3, 1, 2, 4).reshape(BPC, CO, HO, WO)
        )
    return out


LAST_RESULT = None
